# revision 21
# baseline (speedup 1.0000x reference)
"""Distance-aware masking kernel for Trainium2 (8 NeuronCores).

Computes mask[i,j,:] = W2 @ relu(W1 @ [r_i - c_j, |r_i - c_j|] + b1) + b2
for N=4096 nodes, DIM_OUT=8, sharded by rows across 8 cores.

v2 strategy (per core, 512 rows), memory-regime optimized:
  - Output is written to DRAM as float16 (tolerance is 2e-2 of absmax;
    f16 adds ~5e-4) halving HBM write traffic vs f32.
  - d^2 is computed ONCE per 128-row group as a compact [128 rows, J]
    matmul (bf16 triple-split basis for f32-grade accuracy), ACT takes
    sqrt -> d in f16.
  - Per 32-row block, partitions packed p = 4*i_rel + m, ONE PSUM
    accumulation group of two uniform matmuls (both f16, K=128,
    tile_position (0,0) -- mixed configs inside a group hang the PE):
      V matmul:  v_m = alpha_m(i) - g_m(j)   (f16 single, rows 4:128 zero)
      bcast matmul (accumulating): += a_m * d, via per-block G with
        nonzero rows only at this block's 32 d-rows
    so PSUM holds u = a_m*d + v_m directly; DVE applies relu -> f16.
  - TensorEngine mixes 3 hidden units -> 8 outputs with a block-diagonal
    W2 matmul (f16), output partitions q = 8*(i_rel%16) + o.
  - PSUM -> SBUF f16 downcast copy column-split ACT/DVE (GpSimd cannot
    access PSUM); DMA to DRAM scratch laid out [i*8+o, j]; host upcasts,
    transposes to [i, j, o], patches the exact diagonal, adds b2.
"""

import sys

sys.path.insert(0, "/opt/trn_rl_repo")

import numpy as np
import ml_dtypes

N = 4096
N_CORES = 8
ROWS = N // N_CORES          # 512 rows per core
IB = 32                      # i-rows per block (x4 slots = 128 partitions)
NB = ROWS // IB              # 16 blocks
GB = 128                     # i-rows per group (for compact d^2)
NG = ROWS // GB              # 4 groups
J = 512                      # j-tile (columns per tile)
NJ = N // J                  # 8 j-tiles
EPS = 3e-5                   # dist^2 floor; protects sqrt from f32 cancellation
DIM = 3
DIM_OUT = 8

_BF = ml_dtypes.bfloat16
_F16 = np.float16

_CACHE = {}


def _split3(x):
    hi = x.astype(_BF)
    r = x - hi.astype(np.float32)
    mid = r.astype(_BF)
    lo = (r - mid.astype(np.float32)).astype(_BF)
    return hi, mid, lo


def _split2(x):
    hi = x.astype(_BF)
    lo = (x - hi.astype(np.float32)).astype(_BF)
    return hi, lo


def _build_program():
    """Build + compile the SPMD Bass program once (shapes are static)."""
    import concourse.bass as bass  # noqa: F401
    import concourse.mybir as mybir
    import concourse.tile as tile
    from concourse import bacc

    nc = bacc.Bacc("TRN2", target_bir_lowering=False, num_devices=N_CORES)

    f32 = mybir.dt.float32
    f16 = mybir.dt.float16
    bf16 = mybir.dt.bfloat16

    s_lhsT = nc.dram_tensor("s_lhsT", [30, ROWS], bf16, kind="ExternalInput").ap()
    s_rhs = nc.dram_tensor("s_rhs", [30, N], bf16, kind="ExternalInput").ap()
    v_lhsT = nc.dram_tensor("v_lhsT", [128, NB * 128], f16, kind="ExternalInput").ap()
    v_rhs = nc.dram_tensor("v_rhs", [128, N], f16, kind="ExternalInput").ap()
    g128 = nc.dram_tensor("g128", [128, 4 * 128], f16, kind="ExternalInput").ap()
    mixw = nc.dram_tensor("mixw", [128, 128], f16, kind="ExternalInput").ap()
    scratch = nc.dram_tensor("scratch", [N, N], f16, kind="ExternalOutput").ap()

    with tile.TileContext(nc) as tc:
        with tc.tile_pool(name="const", bufs=1) as cp, \
             tc.tile_pool(name="dtile", bufs=2) as dp, \
             tc.tile_pool(name="htile", bufs=3) as hp, \
             tc.tile_pool(name="outp", bufs=6) as op, \
             tc.tile_pool(name="psd", bufs=2, space="PSUM") as psd, \
             tc.tile_pool(name="psv", bufs=2, space="PSUM") as psv, \
             tc.tile_pool(name="psm", bufs=2, space="PSUM") as psm:

            t_s_lhsT = cp.tile([30, ROWS], bf16, tag="t_s_lhsT")
            nc.sync.dma_start(t_s_lhsT[:], s_lhsT)
            t_s_rhs = cp.tile([30, N], bf16, tag="t_s_rhs")
            nc.sync.dma_start(t_s_rhs[:], s_rhs)
            t_v_lhsT = cp.tile([128, NB * 128], f16, tag="t_v_lhsT")
            nc.sync.dma_start(t_v_lhsT[:], v_lhsT)
            t_v_rhs = cp.tile([128, N], f16, tag="t_v_rhs")
            nc.sync.dma_start(t_v_rhs[:], v_rhs)
            t_g128 = cp.tile([128, 4 * 128], f16, tag="t_g128")
            nc.sync.dma_start(t_g128[:], g128)
            t_mixw = cp.tile([128, 128], f16, tag="t_mixw")
            nc.sync.dma_start(t_mixw[:], mixw)

            for g in range(NG):
                gcol = slice(g * GB, (g + 1) * GB)
                for jt in range(NJ):
                    jcol = slice(jt * J, (jt + 1) * J)

                    # compact squared distance for 128 rows
                    ps_d = psd.tile([128, J], f32, tag="ps_d")
                    nc.tensor.matmul(
                        ps_d[:], t_s_lhsT[:, gcol], t_s_rhs[:, jcol],
                        start=True, stop=True,
                    )
                    t_d = dp.tile([128, J], f16, tag="t_d")
                    nc.scalar.activation(
                        t_d[:], ps_d[:], mybir.ActivationFunctionType.Sqrt
                    )

                    for bb in range(GB // IB):
                        b = g * (GB // IB) + bb
                        lcol = slice(b * 128, b * 128 + 128)

                        ps_v = psv.tile([128, J], f32, tag="ps_v")
                        nc.tensor.matmul(
                            ps_v[:], t_v_lhsT[:, lcol], t_v_rhs[:, jcol],
                            start=True, stop=False,
                        )
                        # u = a_m * d + v_m: accumulate broadcast of d via
                        # a K=128 matmul whose G is nonzero only on this
                        # block's 32 d-rows. Both group members are f16,
                        # K=128, tile_position (0,0) -- uniform PE config.
                        nc.tensor.matmul(
                            ps_v[:], t_g128[:, bb * 128:(bb + 1) * 128],
                            t_d[:],
                            start=False, stop=True,
                        )

                        t_h = hp.tile([128, J], f16, tag="t_h")
                        nc.vector.tensor_scalar_max(t_h[:], ps_v[:], 0.0)

                        ps_o = psm.tile([128, 2 * J], f32, tag="ps_o")
                        for w in range(2):
                            nc.tensor.matmul(
                                ps_o[:, w * J:(w + 1) * J],
                                t_mixw[64 * w:64 * w + 64, :],
                                t_h[64 * w:64 * w + 64, :],
                                start=True, stop=True,
                            )
                        # GPSIMD cannot touch PSUM; split the f32->f16 copy
                        # between ACT (larger share) and DVE (which also ran
                        # the relu).
                        t_o = op.tile([128, 2 * J], f16, tag="t_o")
                        CS = 720
                        nc.scalar.copy(t_o[:, 0:CS], ps_o[:, 0:CS])
                        nc.vector.tensor_copy(t_o[:, CS:2 * J], ps_o[:, CS:2 * J])

                        # scratch rows r = 256*b + 128*w + q  <->  sbuf [q, w*J+j]
                        row0 = b * IB * DIM_OUT
                        dview = scratch[row0:row0 + 256, jcol].rearrange(
                            "(w q) j -> q w j", w=2
                        )
                        nc.sync.dma_start(
                            dview, t_o[:].rearrange("q (w j) -> q w j", w=2)
                        )

    nc.compile()
    return nc


def _host_inputs(node_coords, W1, b1, W2, b2):
    """Build per-core input maps (all small host-side numpy work)."""
    coords = node_coords.astype(np.float32)
    W1 = W1.astype(np.float32)
    b1 = b1.astype(np.float32)
    W2 = W2.astype(np.float32)

    a = W1[:, 3]                       # [3] dist coefficients
    Wc = W1[:, :3]                     # [3,3] coord coefficients
    g = coords @ Wc.T                  # [N,3]  g_m(j)
    c2 = (coords * coords).sum(1)      # [N]

    # ---- shared rhs bases ----
    s_base_r = np.zeros((5, N), np.float32)
    s_base_r[0:3] = coords.T
    s_base_r[3] = c2
    s_base_r[4] = 1.0

    v_base_r = np.zeros((128, N), np.float32)
    v_base_r[0] = 1.0
    v_base_r[1:4] = g.T

    Rh, Rm, Rl = _split3(s_base_r)

    # pair order: big (hh) terms first so cancellation happens early
    s_rhs = np.vstack([Rh, Rm, Rh, Rl, Rh, Rm])       # [30, 4096]
    v_rhs = v_base_r                                   # [128, 4096] f16 single

    # ---- mix weights (block-diagonal W2), two identical 64-row windows ----
    mixw = np.zeros((128, 128), np.float32)
    for w in range(2):
        for di in range(16):
            for m in range(3):
                mixw[64 * w + 4 * di + m, 8 * di:8 * di + 8] = W2[:, m]

    # ---- broadcast matrices, one [128,128] per block bb: nonzero only on
    #      rows 32*bb..32*bb+32, mapping d row k -> slot 4*(k%32)+m ----
    g128 = np.zeros((128, 4 * 128), np.float32)
    for bb in range(4):
        for k in range(32 * bb, 32 * bb + 32):
            for m in range(3):
                g128[k, bb * 128 + 4 * (k % 32) + m] = a[m]

    in_maps = []
    for c in range(N_CORES):
        r = coords[c * ROWS:(c + 1) * ROWS]          # [512,3]
        r2 = (r * r).sum(1)                          # [512]
        alpha = r @ Wc.T + b1                        # [512,3]

        # compact S lhs: col = local row index i
        s_base_l = np.zeros((5, ROWS), np.float32)
        s_base_l[0:3] = -2.0 * r.T
        s_base_l[3] = 1.0
        s_base_l[4] = r2 + EPS

        # V lhs: col = b*128 + 4*(i%32) + m
        i_idx = np.arange(ROWS)
        col = (i_idx // IB) * 128 + 4 * (i_idx % IB)  # [512] base col (m=0)
        v_lhsT = np.zeros((128, NB * 128), np.float32)
        for m in range(3):
            cm = col + m
            v_lhsT[0, cm] = alpha[:, m]
            v_lhsT[m + 1, cm] = -1.0

        Lh, Lm, Ll = _split3(s_base_l)

        s_lhsT = np.vstack([Lh, Lh, Lm, Lh, Ll, Lm])      # [30, 512]

        in_maps.append({
            "s_lhsT": np.ascontiguousarray(s_lhsT),
            "s_rhs": np.ascontiguousarray(s_rhs),
            "v_lhsT": v_lhsT.astype(_F16),
            "v_rhs": v_rhs.astype(_F16),
            "g128": g128.astype(_F16),
            "mixw": mixw.astype(_F16),
        })
    return in_maps


def kernel(node_coords, W1, b1, W2, b2):
    from concourse.bass_utils import run_bass_kernel_spmd

    if "nc" not in _CACHE:
        _CACHE["nc"] = _build_program()
    nc = _CACHE["nc"]

    in_maps = _host_inputs(node_coords, W1, b1, W2, b2)
    res = run_bass_kernel_spmd(nc, in_maps, core_ids=list(range(N_CORES)))
    _CACHE["last_res"] = res

    out = np.empty((N, N, DIM_OUT), np.float32)
    for c in range(N_CORES):
        sc = res.results[c]["scratch"]                   # [4096, 4096] f16
        blk = sc.reshape(ROWS, DIM_OUT, N).transpose(0, 2, 1)
        out[c * ROWS:(c + 1) * ROWS] = blk.astype(np.float32)

    # b2 is handled here (the device mix omits it)
    if np.any(b2):
        out += b2.astype(np.float32)

    # exact diagonal (pairwise features are exactly zero there; the device
    # path has an eps floor under the sqrt, so patch on host)
    h_diag = np.maximum(b1.astype(np.float32), 0.0)
    diag = W2.astype(np.float32) @ h_diag + b2.astype(np.float32)
    idx = np.arange(N)
    out[idx, idx, :] = diag

    return out
